# revision 14
# baseline (speedup 1.0000x reference)
"""FPS (npoint=2) Bass kernel v4: grouped finales + batched centroid gather.

Structure per core (8 batches, planes [128,2048] fp32):
- All 24 plane DMAs issued up front, ordered y0-3, z0, y4, z1, y5, z2,
  y6, z3, y7, z4, z5, x0, z6, x1, z7, x2..x7 (y early for the argmax
  groups, z next for s1, x just-in-time for the dist tail).
- Group finales (4 batches wide): one PE-transpose pair + reduce +
  is_eq*cand + reduce per group instead of per batch; ONE indirect DMA
  gathers all 12 centroid coords per group (selection-matrix matmul
  builds the 12 flat offsets).
- Compute is in-place on the plane tiles: sy=Sq(y-cy)->ty,
  sz=Sq(z-cz)->tz, s1=sy+sz->tz (G), sx=Sq(x-cx)->tx,
  s2=s1+sx->tx (V cols 0:CV, G rest), Max8/MaxIndex on tx.
- Dist finale: one 8-wide group at the end.
"""

import os

import numpy as np

import concourse.bacc as bacc
import concourse.bass as bass
import concourse.mybir as mybir
from concourse.masks import make_identity
from concourse.tile import TileContext

B = 64
N_CORES = 8
BPC = B // N_CORES
N = 262144
P = 128
COLS = N // P
BIGK = float(N)
GRP = 4  # y-finale group size

F32 = mybir.dt.float32
U32 = mybir.dt.uint32
I32 = mybir.dt.int32
AX = mybir.AxisListType.X
OP = mybir.AluOpType
SQUARE = mybir.ActivationFunctionType.Square
COPY = mybir.ActivationFunctionType.Copy

CV = int(os.environ.get("V_CV", "640"))


def build_nc():
    nc = bacc.Bacc()
    xin = nc.dram_tensor("xyz", [BPC, 3, N], F32, kind="ExternalInput")
    out = nc.dram_tensor("idx", [1, 2 * BPC], I32, kind="ExternalOutput")
    xflat = xin.rearrange("b c n -> (b c n)")[:, None]

    with TileContext(nc) as tc:
        with (
            tc.tile_pool(name="consts", bufs=1) as consts,
            tc.tile_pool(name="yp", bufs=BPC) as yp,
            tc.tile_pool(name="zp", bufs=BPC) as zp,
            tc.tile_pool(name="xp", bufs=BPC) as xp,
            tc.tile_pool(name="sm", bufs=2) as sm,
            tc.tile_pool(name="acc", bufs=1) as acc,
            tc.tile_pool(name="psT", bufs=2, space="PSUM") as psT,
            tc.tile_pool(name="psI", bufs=2, space="PSUM") as psI,
            tc.tile_pool(name="psS", bufs=1, space="PSUM") as psS,
        ):
            # ---- constants ----
            ident = consts.tile([P, P], F32)
            make_identity(nc, ident)
            ones = consts.tile([1, P], F32)
            nc.vector.memset(ones, 1.0)
            revb_i = consts.tile([P, 1], I32)
            nc.gpsimd.iota(revb_i, pattern=[[0, 1]], base=N, channel_multiplier=-COLS)
            revb_f = consts.tile([P, 1], F32)
            nc.vector.tensor_copy(revb_f, revb_i)
            # E[q, p] = (p // 3 == q) selection matrix [GRP, 3*GRP]
            e_j = consts.tile([GRP, 3 * GRP], I32)
            nc.gpsimd.iota(
                e_j.rearrange("q (b c) -> q b c", c=3),
                pattern=[[1, GRP], [0, 3]], base=0, channel_multiplier=0,
            )
            e_p = consts.tile([GRP, 3 * GRP], I32)
            nc.gpsimd.iota(e_p, pattern=[[0, 3 * GRP]], base=0, channel_multiplier=1)
            emat = consts.tile([GRP, 3 * GRP], F32)
            nc.vector.tensor_tensor(emat, e_j, e_p, op=OP.is_equal)
            # bpn[p] = N * p for p in 0..3*GRP (flat offset base within group)
            bpn = consts.tile([3 * GRP, 1], I32)
            nc.gpsimd.iota(bpn, pattern=[[0, 1]], base=0, channel_multiplier=N)

            out_i = acc.tile([1, 2 * BPC], I32)
            dYV8 = acc.tile([P, 8 * BPC], F32)
            dYI8 = acc.tile([P, 8 * BPC], U32)
            dDV8 = acc.tile([P, 8 * BPC], F32)
            dDI8 = acc.tile([P, 8 * BPC], U32)

            def col0(t, lo, hi):
                return t.rearrange("p (b k) -> p b k", k=8)[:, lo:hi, 0]

            tys = [None] * BPC
            tzs = [None] * BPC
            txs = [None] * BPC
            negc_g = [None, None]

            def dma_plane(b, c, pool, store):
                t = pool.tile([P, COLS], F32, tag="t")
                store[b] = t
                nc.sync.dma_start(t, xin[b, c].rearrange("(p m) -> p m", p=P))

            def group_finale(v8, i8, lo, nb, out_lo, tagp):
                """nb-wide finale over batches [lo, lo+nb); returns idxs
                [nb,1] f32 tile of winning global indices; writes out_i."""
                vals = col0(v8, lo, lo + nb)  # [P, nb]
                cand = sm.tile([P, nb], F32, tag=f"cand{tagp}")
                nc.vector.tensor_scalar(
                    out=cand, in0=col0(i8, lo, lo + nb), scalar1=-1.0,
                    scalar2=revb_f, op0=OP.mult, op1=OP.add,
                )
                pt = psT.tile([BPC, 2 * P], F32, tag="pt")
                nc.tensor.transpose(pt[0:nb, 0:P], vals, ident)
                nc.tensor.transpose(pt[0:nb, P : 2 * P], cand, ident)
                rows = sm.tile([BPC, 2 * P], F32, tag=f"rows{tagp}")
                nc.scalar.copy(rows[0:nb], pt[0:nb])
                mx = sm.tile([BPC, 1], F32, tag=f"mx{tagp}")
                nc.vector.tensor_reduce(mx[0:nb], rows[0:nb, 0:P], axis=AX, op=OP.max)
                cands = sm.tile([BPC, P], F32, tag=f"cands{tagp}")
                nc.vector.scalar_tensor_tensor(
                    out=cands[0:nb], in0=rows[0:nb, 0:P], scalar=mx[0:nb, 0:1],
                    in1=rows[0:nb, P : 2 * P], op0=OP.is_equal, op1=OP.mult,
                )
                wc = sm.tile([BPC, 1], F32, tag=f"wc{tagp}")
                nc.vector.tensor_reduce(wc[0:nb], cands[0:nb], axis=AX, op=OP.max)
                idxs = sm.tile([BPC, 1], F32, tag=f"idxs{tagp}")
                nc.vector.tensor_scalar(
                    out=idxs[0:nb], in0=wc[0:nb], scalar1=-1.0, scalar2=BIGK,
                    op0=OP.mult, op1=OP.add,
                )
                pti = psI.tile([1, BPC], F32, tag="pti")
                nc.tensor.transpose(pti[0:1, 0:nb], idxs[0:nb], ident[0:nb, 0:nb])
                nc.scalar.copy(out_i[0:1, out_lo : out_lo + nb], pti[0:1, 0:nb])
                return idxs

            def centroid_group(g, idxs):
                """Gather 3*GRP centroid coords for group g; negc [P, 3*GRP]."""
                m = 3 * GRP
                pofs = psS.tile([m, 1], F32, tag="pofs")
                nc.tensor.matmul(pofs, emat, idxs[0:GRP], start=True, stop=True)
                offs = sm.tile([m, 1], U32, tag="offs")
                # flat = idx0_b + N*(3b+c); partition p = 3*(b-4g)+c
                nc.vector.scalar_tensor_tensor(
                    out=offs, in0=pofs, scalar=float(3 * GRP * g * N), in1=bpn,
                    op0=OP.add, op1=OP.add,
                )
                cg = sm.tile([m, 1], F32, tag="cg")
                nc.gpsimd.indirect_dma_start(
                    out=cg, out_offset=None, in_=xflat,
                    in_offset=bass.IndirectOffsetOnAxis(ap=offs[0:m, 0:1], axis=0),
                )
                pcr = psS.tile([1, m], F32, tag="pcr")
                nc.tensor.transpose(pcr, cg, ident[0:m, 0:m])
                negrow = sm.tile([1, m], F32, tag="negrow")
                nc.scalar.mul(negrow, pcr, -1.0)
                pneg = psS.tile([P, m], F32, tag="pneg")
                nc.tensor.matmul(pneg, ones, negrow, start=True, stop=True)
                negc = sm.tile([P, m], F32, tag=f"negc{g}")
                nc.scalar.copy(negc, pneg)
                negc_g[g] = negc

            def nbias(b, c):
                g = b // GRP
                return negc_g[g][:, 3 * (b - GRP * g) + c : 3 * (b - GRP * g) + c + 1]

            # ---- DMAs up front ----
            for b in range(GRP):
                dma_plane(b, 1, yp, tys)
            order = []
            for b in range(GRP):
                order.append((b, 2))          # z0..z3 interleaved with y4..y7
                order.append((GRP + b, 1))
            order += [(GRP, 2), (GRP + 1, 2), (0, 0), (GRP + 2, 2), (1, 0),
                      (GRP + 3, 2), (2, 0)]
            order += [(b, 0) for b in range(3, BPC)]
            for b, c in order:
                dma_plane(b, c, yp if c == 1 else (zp if c == 2 else xp),
                          tys if c == 1 else (tzs if c == 2 else txs))

            # ---- y scans + grouped finales/centroids + sy/sz/s1 ----
            for g in range(2):
                lo = g * GRP
                for b in range(lo, lo + GRP):
                    ym8 = dYV8[:, 8 * b : 8 * b + 8]
                    nc.vector.max(out=ym8, in_=tys[b])
                    nc.vector.max_index(dYI8[:, 8 * b : 8 * b + 8], ym8, tys[b])
                idxs = group_finale(dYV8, dYI8, lo, GRP, lo, f"y{g}")
                centroid_group(g, idxs)
                for b in range(lo, lo + GRP):
                    nc.scalar.activation(tys[b], tys[b], SQUARE, bias=nbias(b, 1))
                    nc.scalar.activation(tzs[b], tzs[b], SQUARE, bias=nbias(b, 2))
                    nc.gpsimd.tensor_add(tzs[b], tys[b], tzs[b])

            # ---- dist phase per batch ----
            for b in range(BPC):
                tz, tx = tzs[b], txs[b]
                nc.scalar.activation(tx, tx, SQUARE, bias=nbias(b, 0))
                nc.vector.tensor_add(tx[:, 0:CV], tz[:, 0:CV], tx[:, 0:CV])
                nc.gpsimd.tensor_add(tx[:, CV:], tz[:, CV:], tx[:, CV:])
                dm8 = dDV8[:, 8 * b : 8 * b + 8]
                nc.vector.max(out=dm8, in_=tx)
                nc.vector.max_index(dDI8[:, 8 * b : 8 * b + 8], dm8, tx)

            group_finale(dDV8, dDI8, 0, BPC, BPC, "d")

            nc.sync.dma_start(out[:, :], out_i[:, :])

    nc.compile()
    return nc


_NC_CACHE = None


def _get_nc():
    global _NC_CACHE
    if _NC_CACHE is None:
        _NC_CACHE = build_nc()
    return _NC_CACHE


def kernel(xyz: np.ndarray) -> np.ndarray:
    from concourse.bass_utils import run_bass_kernel_spmd

    assert xyz.shape == (1, B, 3, N), xyz.shape
    xyz = np.ascontiguousarray(xyz, dtype=np.float32)
    nc = _get_nc()
    in_maps = [
        {"xyz": np.ascontiguousarray(xyz[0, k * BPC : (k + 1) * BPC])}
        for k in range(N_CORES)
    ]
    res = run_bass_kernel_spmd(nc, in_maps, core_ids=list(range(N_CORES)))
    outs = [res.results[k]["idx"].reshape(2, BPC).T for k in range(N_CORES)]
    return np.concatenate(outs, axis=0).astype(np.int64)
